# revision 13
# baseline (speedup 1.0000x reference)
"""Trainium2 Bass kernel for attention-LSTM decoder (teacher-forced).

Reference computation (per batch element b, S=21 steps):
    Hp = batch_H @ Wi.T                      [B,T,H]   (precomputed once)
    per step s:
        ph    = h @ Wh.T + bh                [B,H]
        e     = tanh(Hp + ph[:,None,:]) @ Ws [B,T]
        alpha = softmax(e, axis=T)
        ctx   = alpha @ batch_H              [B,D]
        gates = [ctx,oh] @ Wih.T + bih + h @ Whh.T + bhh
        LSTM pointwise -> h, c
    probs = hs @ Wg.T + bg                   [B,S,C]

Sharding: data-parallel over batch (1024 -> 128 per core x 8 cores),
weights replicated, recurrence local per core.

Layouts (per core, BC=128):
  Hp   resident SBUF [h(4x128 part), t*128+b (8192 free)] bf16, +bh folded
  BH   resident SBUF [b(128 part), t, d] bf16  (ctx matmul moving operand)
  scores: Z = Hp + ph (DVE bcast add, bf16 2x) -> tanh (ACT) ->
          e = Ws.T @ X per 512-block (PE, M=1 matvec, K-chunk accumulated)
  softmax: exp on ACT with accum_out (no max subtraction; |e|<=18 so safe)
  ctx: sum_t diag(expe_t) @ BH_t accumulated in PSUM (PE), normalized by
       1/sum(expe) during PSUM->SBUF copy (per-partition scalar)
  gates: out[b, 4H] = sum_k xT/hT[k].T @ Wcat[k]; bias via constant-1 row
  sigmoid(x) = 0.5*tanh(x/2)+0.5 derived on DVE so ACT uses one table set
"""

import numpy as np
import ml_dtypes

import sys

sys.path.insert(0, "/opt/trn_rl_repo")

import concourse.bass as bass  # noqa: E402
import concourse.mybir as mybir  # noqa: E402
import concourse.tile as tile  # noqa: E402
from concourse import bacc  # noqa: E402
from concourse.bass_utils import run_bass_kernel_spmd  # noqa: E402

BF16 = mybir.dt.bfloat16
F32 = mybir.dt.float32
AF = mybir.ActivationFunctionType
ALU = mybir.AluOpType

B, T, D, H, C, S = 1024, 64, 512, 512, 96, 21
NCORES = 8
BC = B // NCORES  # 128 batch per core
HK = H // 128  # 4 h chunks
DK = D // 128  # 4 d chunks
NTB = T * BC  # 8192 flattened (t,b), t-major
XDIM = 640  # ctx(512) + onehot(96) + bias-one(1) + pad(31)
XK = XDIM // 128  # 5
NE8 = 8  # eighths of the (t,b) range per step (8 t each)
E8 = NTB // NE8  # 1024 flat elements per eighth

_CACHE = {}


def _build():
    """Build the Bass program (single NEFF, SPMD across 8 cores)."""
    nc = bacc.Bacc(
        "TRN2",
        target_bir_lowering=False,
        debug=False,
        enable_asserts=False,
        num_devices=1,
    )

    # ---- DRAM I/O (per-core shapes) ----
    d_bht = nc.dram_tensor("bht", [D, T, BC], BF16, kind="ExternalInput").ap()
    d_bhres = nc.dram_tensor("bhres", [BC, T, D], BF16, kind="ExternalInput").ap()
    d_wit = nc.dram_tensor("wit", [DK, 128, H], BF16, kind="ExternalInput").ap()
    d_wcat = nc.dram_tensor("wcat", [9, 128, 4 * H], BF16, kind="ExternalInput").ap()
    d_wht = nc.dram_tensor("wht", [HK, 128, H], BF16, kind="ExternalInput").ap()
    d_wgt = nc.dram_tensor("wgt", [HK, 128, C], BF16, kind="ExternalInput").ap()
    d_wsp = nc.dram_tensor("wsp", [128, HK], BF16, kind="ExternalInput").ap()
    d_bhb = nc.dram_tensor("bhb", [128, HK], F32, kind="ExternalInput").ap()
    d_oht = nc.dram_tensor("oht", [128, S, BC], BF16, kind="ExternalInput").ap()
    d_bg = nc.dram_tensor("bgr", [1, C], BF16, kind="ExternalInput").ap()
    d_ones = nc.dram_tensor("onesr", [1, 128], BF16, kind="ExternalInput").ap()
    d_idbf = nc.dram_tensor("idbf", [128, 128], BF16, kind="ExternalInput").ap()
    d_idhf = nc.dram_tensor("idhf", [128, 128], F32, kind="ExternalInput").ap()
    d_out = nc.dram_tensor("probs", [BC, S, C], F32, kind="ExternalOutput").ap()

    with tile.TileContext(nc) as tc:
        import contextlib

        es = contextlib.ExitStack()
        with es:
            singles = es.enter_context(tc.tile_pool(name="singles", bufs=1))

            # ---- resident tensors ----
            HPs = [singles.tile([128, NTB], BF16, tag=f"hp{i}", name=f"hp{i}") for i in range(HK)]
            BHR = singles.tile([BC, T, D], BF16, tag="bhres")
            WHT = singles.tile([128, HK, H], BF16, tag="wht")
            WGT = singles.tile([128, HK, C], BF16, tag="wgt")
            WSP = singles.tile([128, HK], BF16, tag="wsp")
            BHB = singles.tile([128, HK], F32, tag="bhb")
            OHT = singles.tile([128, S, BC], BF16, tag="oht")
            Bb = singles.tile([1, C], BF16, tag="bg")
            ONESR = singles.tile([1, 128], BF16, tag="ones")
            IDBF = singles.tile([128, 128], BF16, tag="idbf")
            IDHF = singles.tile([128, 128], F32, tag="idhf")
            ESB = singles.tile([BC, T], F32, tag="esb")
            SUMS = singles.tile([BC, NE8], F32, tag="sums")
            RS = singles.tile([BC, 1], F32, tag="rs")
            CS = singles.tile([BC, H], F32, tag="cstate")

            nc.sync.dma_start(out=BHR, in_=d_bhres)
            for k in range(HK):
                nc.sync.dma_start(out=WHT[:, k, :], in_=d_wht[k])
                nc.sync.dma_start(out=WGT[:, k, :], in_=d_wgt[k])
            nc.sync.dma_start(out=WSP, in_=d_wsp)
            nc.sync.dma_start(out=BHB, in_=d_bhb)
            nc.sync.dma_start(out=OHT, in_=d_oht)
            nc.sync.dma_start(out=Bb, in_=d_bg)
            nc.sync.dma_start(out=ONESR, in_=d_ones)
            nc.sync.dma_start(out=IDBF, in_=d_idbf)
            nc.sync.dma_start(out=IDHF, in_=d_idhf)

            nc.vector.memset(CS, 0.0)

            # ---- preamble: Hp = batch_H @ Wi.T (+bh), into [h, (t,b)] ----
            with tc.tile_pool(name="bhtp", bufs=10) as bhtp, tc.tile_pool(
                name="hp_ps", bufs=4, space="PSUM"
            ) as hp_ps_pool:
                WIT = bhtp.tile([128, DK, H], BF16, tag="wit", bufs=1)
                for k in range(DK):
                    nc.sync.dma_start(out=WIT[:, k, :], in_=d_wit[k])
                for nb in range(NTB // 512):  # 16 blocks of 512 (t,b)
                    rhs_tiles = []
                    for kd in range(DK):
                        bt = bhtp.tile([128, 512], BF16, tag="bht_in")
                        nc.sync.dma_start(
                            out=bt,
                            in_=d_bht[kd * 128 : (kd + 1) * 128, 4 * nb : 4 * nb + 4, :],
                        )
                        rhs_tiles.append(bt)
                    for mh in range(HK):
                        ps = hp_ps_pool.tile([128, 512], F32, tag="hp_ps")
                        for kd in range(DK):
                            nc.tensor.matmul(
                                ps,
                                WIT[:, kd, mh * 128 : (mh + 1) * 128],
                                rhs_tiles[kd],
                                start=(kd == 0),
                                stop=(kd == DK - 1),
                            )
                        # fold bh while copying PSUM->SBUF (bf16 out)
                        nc.vector.tensor_scalar(
                            out=HPs[mh][:, nb * 512 : (nb + 1) * 512],
                            in0=ps,
                            scalar1=BHB[:, mh : mh + 1],
                            scalar2=None,
                            op0=ALU.add,
                        )

            # ---- step-loop pools ----
            xpool = es.enter_context(tc.tile_pool(name="xpool", bufs=2))
            wstrm = es.enter_context(tc.tile_pool(name="wstrm", bufs=5))
            estp = es.enter_context(tc.tile_pool(name="estp", bufs=2))
            dpool = es.enter_context(tc.tile_pool(name="dpool", bufs=3))
            phpool = es.enter_context(tc.tile_pool(name="phpool", bufs=2))
            htpool = es.enter_context(tc.tile_pool(name="htpool", bufs=2))
            actp = es.enter_context(tc.tile_pool(name="actp", bufs=2))
            fpool = es.enter_context(tc.tile_pool(name="fpool", bufs=2))
            ctxp = es.enter_context(tc.tile_pool(name="ctxp", bufs=2))
            xtp = es.enter_context(tc.tile_pool(name="xtp", bufs=1))

            e_psp = es.enter_context(tc.tile_pool(name="e_ps", bufs=2, space="PSUM"))
            ctx_psp = es.enter_context(
                tc.tile_pool(name="ctx_ps", bufs=1, space="PSUM")
            )
            g_psp = es.enter_context(tc.tile_pool(name="g_ps", bufs=1, space="PSUM"))
            sm_psp = es.enter_context(tc.tile_pool(name="sm_ps", bufs=1, space="PSUM"))

            # initial ph = 0 (h0 = 0), initial hT = 0
            ph_sb = phpool.tile([128, HK, BC], BF16, tag="ph")
            nc.vector.memset(ph_sb, 0.0)
            hT = htpool.tile([128, HK, BC], BF16, tag="ht")
            nc.vector.memset(hT, 0.0)

            for s in range(S):
                # -- stream gate weights for this step (hidden under tanh) --
                wc = []
                for k in range(9):
                    wt = wstrm.tile([128, 4 * H], BF16, tag="wcat")
                    nc.sync.dma_start(out=wt, in_=d_wcat[k])
                    wc.append(wt)

                ctx_ps = ctx_psp.tile([128, D], F32, tag="ctx")

                # -- attention scores + online ctx accumulation --
                e_ps = None
                for e8 in range(NE8):  # 8 t's per eighth
                    xq = xpool.tile([128, HK, E8], BF16, tag="xq")
                    for hc in range(HK):
                        ph_b = (
                            ph_sb[:, hc, :]
                            .unsqueeze(1)
                            .broadcast_to([128, E8 // BC, BC])
                        )
                        nc.vector.tensor_tensor(
                            out=xq[:, hc, :].rearrange(
                                "p (t b) -> p t b", b=BC
                            ),
                            in0=HPs[hc][:, e8 * E8 : (e8 + 1) * E8].rearrange(
                                "p (t b) -> p t b", b=BC
                            ),
                            in1=ph_b,
                            op=ALU.add,
                        )
                    nc.scalar.activation(
                        out=xq[:, :, :], in_=xq[:, :, :], func=AF.Tanh
                    )
                    # e blocks: Ws stationary (1-col ldweights), X streaming
                    # rhs free-reordered (b,t) so the row scatters cleanly
                    for j in range(E8 // 512):  # 2 blocks of 4 t
                        e_ps = e_psp.tile([128, 512], F32, tag="e_ps")
                        for hc in range(HK):
                            nc.tensor.matmul(
                                e_ps[0:1, :],
                                WSP[:, hc : hc + 1],
                                xq[:, hc, j * 512 : (j + 1) * 512].rearrange(
                                    "p (t b) -> p b t", b=BC
                                ),
                                start=(hc == 0),
                                stop=(hc == HK - 1),
                            )
                        est = estp.tile([1, 512], F32, tag="est", name="est")
                        nc.vector.tensor_copy(out=est, in_=e_ps[0:1, :])
                        nc.sync.dma_start(
                            out=ESB[:, e8 * 8 + 4 * j : e8 * 8 + 4 * j + 4],
                            in_=est,
                        )
                    # exp + partial sum for this eighth
                    nc.scalar.activation(
                        out=ESB[:, e8 * 8 : e8 * 8 + 8],
                        in_=ESB[:, e8 * 8 : e8 * 8 + 8],
                        func=AF.Exp,
                        accum_out=SUMS[:, e8 : e8 + 1],
                    )
                    # online ctx: += diag(expe_t) @ BH_t (diag built on GPSIMD)
                    for tl in range(8):
                        t = e8 * 8 + tl
                        dg = dpool.tile([128, 128], BF16, tag="diag")
                        nc.gpsimd.tensor_scalar(
                            out=dg,
                            in0=IDBF,
                            scalar1=ESB[:, t : t + 1],
                            scalar2=None,
                            op0=ALU.mult,
                        )
                        nc.tensor.matmul(
                            ctx_ps,
                            dg,
                            BHR[:, t, :],
                            start=(t == 0),
                            stop=(t == T - 1),
                        )

                # -- softmax denominator -> rs = 1/sum --
                nc.vector.tensor_reduce(
                    out=RS, in_=SUMS, axis=mybir.AxisListType.X, op=ALU.add
                )
                nc.vector.reciprocal(out=RS, in_=RS)

                # -- ctx -> SBUF (normalized), transpose to [d, b] --
                ctx_sb = ctxp.tile([128, D], BF16, tag="ctx_sb")
                nc.vector.tensor_scalar(
                    out=ctx_sb,
                    in0=ctx_ps,
                    scalar1=RS,
                    scalar2=None,
                    op0=ALU.mult,
                )
                xT = xtp.tile([128, DK, BC], BF16, tag="xT")
                for md in range(DK):
                    tp = sm_psp.tile([128, 512], BF16, tag="small", name="tpb")
                    nc.tensor.transpose(
                        tp[:, 0:128], ctx_sb[:, md * 128 : (md + 1) * 128], IDBF
                    )
                    nc.vector.tensor_copy(out=xT[:, md, :], in_=tp[:, 0:128])

                # -- gates = sum_k lhsT_k.T @ wcat_k  [b, 4H(i,f,o,g)] --
                g_ps = g_psp.tile([128, 4 * H], F32, tag="gates")
                lhs = [xT[:, k, :] for k in range(DK)] + [OHT[:, s, :]] + [
                    hT[:, k, :] for k in range(HK)
                ]
                for k in range(9):
                    for ng in range(4):
                        nc.tensor.matmul(
                            g_ps[:, ng * 512 : (ng + 1) * 512],
                            lhs[k],
                            wc[k][:, ng * 512 : (ng + 1) * 512],
                            start=(k == 0),
                            stop=(k == 8),
                        )

                # -- LSTM pointwise; sigmoid via tanh --
                tifo = actp.tile([128, 3 * 512], BF16, tag="tifo", bufs=1)
                nc.scalar.activation(
                    out=tifo, in_=g_ps[:, 0 : 3 * 512], func=AF.Tanh, scale=0.5
                )
                tg = actp.tile([128, 512], BF16, tag="tg")
                nc.scalar.activation(
                    out=tg, in_=g_ps[:, 3 * 512 : 4 * 512], func=AF.Tanh
                )
                p1 = fpool.tile([128, 512], F32, tag="pw")
                nc.vector.scalar_tensor_tensor(
                    out=p1,
                    in0=tifo[:, 512:1024],
                    scalar=1.0,
                    in1=CS,
                    op0=ALU.add,
                    op1=ALU.mult,
                )
                p2 = fpool.tile([128, 512], F32, tag="pw")
                nc.vector.scalar_tensor_tensor(
                    out=p2,
                    in0=tifo[:, 0:512],
                    scalar=1.0,
                    in1=tg,
                    op0=ALU.add,
                    op1=ALU.mult,
                )
                # p1 <- p1 + p2 = 2*c_new
                nc.vector.tensor_tensor(out=p1, in0=p1, in1=p2, op=ALU.add)
                nc.vector.tensor_scalar(
                    out=CS, in0=p1, scalar1=0.5, scalar2=None, op0=ALU.mult
                )
                tc2 = actp.tile([128, 512], BF16, tag="tc2")
                nc.scalar.activation(out=tc2, in_=p1, func=AF.Tanh, scale=0.5)
                h2x2 = fpool.tile([128, 512], F32, tag="h2")
                nc.vector.scalar_tensor_tensor(
                    out=h2x2,
                    in0=tifo[:, 1024:1536],
                    scalar=1.0,
                    in1=tc2,
                    op0=ALU.add,
                    op1=ALU.mult,
                )

                # -- hT = 0.5 * h2x2.T (transpose bakes the 0.5) --
                hT = htpool.tile([128, HK, BC], BF16, tag="ht")
                for mo in range(HK):
                    tp = sm_psp.tile([128, 512], F32, tag="small")
                    nc.tensor.transpose(
                        tp[:, 0:128], h2x2[:, mo * 128 : (mo + 1) * 128], IDHF
                    )
                    nc.vector.tensor_copy(out=hT[:, mo, :], in_=tp[:, 0:128])

                # -- probs_s = h @ Wg.T + bg -> DRAM --
                pr = sm_psp.tile([128, 512], F32, tag="small")
                for k in range(HK):
                    nc.tensor.matmul(
                        pr[:, 0:C],
                        hT[:, k, :],
                        WGT[:, k, :],
                        start=(k == 0),
                        stop=False,
                    )
                nc.tensor.matmul(
                    pr[:, 0:C], ONESR, Bb, start=False, stop=True
                )
                pr_sb = ctxp.tile([128, C], F32, tag="pr_sb", name="pr_sb", bufs=1)
                nc.vector.tensor_copy(out=pr_sb, in_=pr[:, 0:C])
                nc.sync.dma_start(out=d_out[:, s, :], in_=pr_sb)

                # -- ph for next step: ph = Wh @ h, [hout, b] --
                if s + 1 < S:
                    php = sm_psp.tile([128, 512], F32, tag="small")
                    for mo in range(HK):
                        for k in range(HK):
                            nc.tensor.matmul(
                                php[:, mo * 128 : (mo + 1) * 128],
                                WHT[:, k, mo * 128 : (mo + 1) * 128],
                                hT[:, k, :],
                                start=(k == 0),
                                stop=(k == HK - 1),
                            )
                    ph_sb = phpool.tile([128, HK, BC], BF16, tag="ph")
                    nc.vector.tensor_copy(
                        out=ph_sb, in_=php.rearrange("p (k b) -> p k b", b=BC)
                    )

    nc.compile()
    return nc


def _prep(inputs):
    """Host-side layout prep (casts/transposes/onehots). Returns in_maps."""
    bf = ml_dtypes.bfloat16
    batch_H = np.asarray(inputs["batch_H"], np.float32)
    text = np.asarray(inputs["text"])
    Wi = np.asarray(inputs["Wi"], np.float32)
    Wh = np.asarray(inputs["Wh"], np.float32)
    bh = np.asarray(inputs["bh"], np.float32)
    Ws = np.asarray(inputs["Ws"], np.float32)
    Wih = np.asarray(inputs["Wih"], np.float32)
    Whh = np.asarray(inputs["Whh"], np.float32)
    bih = np.asarray(inputs["bih"], np.float32)
    bhh = np.asarray(inputs["bhh"], np.float32)
    Wg = np.asarray(inputs["Wg"], np.float32)
    bg = np.asarray(inputs["bg"], np.float32)

    bht_full = np.ascontiguousarray(batch_H.transpose(2, 1, 0)).astype(bf)  # [D,T,B]
    bhres_full = batch_H.astype(bf)  # [B,T,D]

    wit = np.ascontiguousarray(Wi.T).reshape(DK, 128, H).astype(bf)
    # hT is stored as 2*h.T (transpose can't scale); fold 0.5 into all
    # weights that consume hT
    wht = np.ascontiguousarray(0.5 * Wh.T).reshape(HK, 128, H).astype(bf)
    wgt = np.ascontiguousarray(0.5 * Wg.T).reshape(HK, 128, C).astype(bf)
    wsp = np.ascontiguousarray(Ws[0].reshape(HK, 128).T).astype(bf)  # [128, HK]
    bhb = np.ascontiguousarray(bh.reshape(HK, 128).T).astype(np.float32)

    # gate permutation: torch (i,f,g,o) -> ours (i,f,o,g)
    perm = np.concatenate(
        [np.arange(0, 1024), np.arange(1536, 2048), np.arange(1024, 1536)]
    )
    Wihp = Wih[perm]
    Whhp = Whh[perm]
    biasp = (bih + bhh)[perm]
    xmat = np.zeros((XDIM, 4 * H), np.float32)
    xmat[0:D] = Wihp[:, 0:D].T
    xmat[D : D + C] = Wihp[:, D : D + C].T
    xmat[D + C] = biasp
    wcat = np.concatenate([xmat, 0.5 * Whhp.T], axis=0)  # [1152, 2048]
    wcat = np.ascontiguousarray(wcat).reshape(9, 128, 4 * H).astype(bf)

    # one-hot (transposed, with constant-1 row at 96) per core
    oht_full = np.zeros((128, S, B), np.float32)
    cb = np.arange(C)
    for s in range(S):
        oht_full[:C, s, :] = (text[:, s][None, :] == cb[:, None]).astype(np.float32)
    oht_full[C, :, :] = 1.0
    oht_full = oht_full.astype(bf)

    bgr = bg.reshape(1, C).astype(bf)
    onesr = np.ones((1, 128), bf)
    idbf = np.eye(128, dtype=np.float32).astype(bf)
    idhf = np.eye(128, dtype=np.float32)

    in_maps = []
    for c in range(NCORES):
        sl = slice(c * BC, (c + 1) * BC)
        in_maps.append(
            {
                "bht": np.ascontiguousarray(bht_full[:, :, sl]),
                "bhres": np.ascontiguousarray(bhres_full[sl]),
                "wit": wit,
                "wcat": wcat,
                "wht": wht,
                "wgt": wgt,
                "wsp": wsp,
                "bhb": bhb,
                "oht": np.ascontiguousarray(oht_full[:, :, sl]),
                "bgr": bgr,
                "onesr": onesr,
                "idbf": idbf,
                "idhf": idhf,
            }
        )
    return in_maps


def get_nc():
    if "nc" not in _CACHE:
        _CACHE["nc"] = _build()
    return _CACHE["nc"]


def kernel(trace=False, **inputs) -> np.ndarray:
    nc = get_nc()
    in_maps = _prep(inputs)
    res = run_bass_kernel_spmd(
        nc, in_maps, core_ids=list(range(NCORES)), trace=trace
    )
    out = np.concatenate([r["probs"] for r in res.results], axis=0)
    _CACHE["last_results"] = res
    return out


# revision 15
# speedup vs baseline: 2.7071x; 2.7071x over previous
"""Trainium2 Bass kernel for attention-LSTM decoder (teacher-forced).

Reference computation (per batch element b, S=21 steps):
    Hp = batch_H @ Wi.T                      [B,T,H]   (precomputed once)
    per step s:
        ph    = h @ Wh.T + bh                [B,H]
        e     = tanh(Hp + ph[:,None,:]) @ Ws [B,T]
        alpha = softmax(e, axis=T)
        ctx   = alpha @ batch_H              [B,D]
        gates = [ctx,oh] @ Wih.T + bih + h @ Whh.T + bhh
        LSTM pointwise -> h, c
    probs = hs @ Wg.T + bg                   [B,S,C]

Sharding: data-parallel over batch (1024 -> 128 per core x 8 cores),
weights replicated, recurrence local per core.

Layouts (per core, BC=128):
  Hp   resident SBUF [h(4x128 part), t*128+b (8192 free)] bf16, +bh folded
  BH   resident SBUF [b(128 part), t, d] bf16  (ctx matmul moving operand)
  scores: Z = Hp + ph (DVE bcast add, bf16 2x) -> tanh (ACT) ->
          e = Ws.T @ X per 512-block (PE, M=1 matvec, K-chunk accumulated)
  softmax: exp on ACT with accum_out (no max subtraction; |e|<=18 so safe)
  ctx: sum_t diag(expe_t) @ BH_t accumulated in PSUM (PE), normalized by
       1/sum(expe) during PSUM->SBUF copy (per-partition scalar)
  gates: out[b, 4H] = sum_k xT/hT[k].T @ Wcat[k]; bias via constant-1 row
  sigmoid(x) = 0.5*tanh(x/2)+0.5 derived on DVE so ACT uses one table set
"""

import numpy as np
import ml_dtypes

import sys

sys.path.insert(0, "/opt/trn_rl_repo")

import concourse.bass as bass  # noqa: E402
import concourse.mybir as mybir  # noqa: E402
import concourse.tile as tile  # noqa: E402
from concourse import bacc  # noqa: E402
from concourse.bass_utils import run_bass_kernel_spmd  # noqa: E402

BF16 = mybir.dt.bfloat16
F32 = mybir.dt.float32
AF = mybir.ActivationFunctionType
ALU = mybir.AluOpType

B, T, D, H, C, S = 1024, 64, 512, 512, 96, 21
NCORES = 8
BC = B // NCORES  # 128 batch per core
HK = H // 128  # 4 h chunks
DK = D // 128  # 4 d chunks
NTB = T * BC  # 8192 flattened (t,b), t-major
XDIM = 640  # ctx(512) + onehot(96) + bias-one(1) + pad(31)
XK = XDIM // 128  # 5
NE8 = 8  # eighths of the (t,b) range per step (8 t each)
E8 = NTB // NE8  # 1024 flat elements per eighth

_CACHE = {}


def _build():
    """Build the Bass program (single NEFF, SPMD across 8 cores)."""
    nc = bacc.Bacc(
        "TRN2",
        target_bir_lowering=False,
        debug=False,
        enable_asserts=False,
        num_devices=1,
    )

    # ---- DRAM I/O (per-core shapes) ----
    d_bht = nc.dram_tensor("bht", [D, T, BC], BF16, kind="ExternalInput").ap()
    d_bhres = nc.dram_tensor("bhres", [BC, T, D], BF16, kind="ExternalInput").ap()
    d_wit = nc.dram_tensor("wit", [DK, 128, H], BF16, kind="ExternalInput").ap()
    d_wcat = nc.dram_tensor("wcat", [9, 128, 4 * H], BF16, kind="ExternalInput").ap()
    d_wht = nc.dram_tensor("wht", [HK, 128, H], BF16, kind="ExternalInput").ap()
    d_wgt = nc.dram_tensor("wgt", [HK, 128, C], BF16, kind="ExternalInput").ap()
    d_wsp = nc.dram_tensor("wsp", [128, HK], BF16, kind="ExternalInput").ap()
    d_bhb = nc.dram_tensor("bhb", [128, HK], F32, kind="ExternalInput").ap()
    d_oht = nc.dram_tensor("oht", [128, S, BC], BF16, kind="ExternalInput").ap()
    d_bg = nc.dram_tensor("bgr", [1, C], BF16, kind="ExternalInput").ap()
    d_ones = nc.dram_tensor("onesr", [1, 128], BF16, kind="ExternalInput").ap()
    d_idbf = nc.dram_tensor("idbf", [128, 128], BF16, kind="ExternalInput").ap()
    d_idhf = nc.dram_tensor("idhf", [128, 128], F32, kind="ExternalInput").ap()
    d_out = nc.dram_tensor("probs", [BC, S, C], F32, kind="ExternalOutput").ap()

    with tile.TileContext(nc) as tc:
        import contextlib

        es = contextlib.ExitStack()
        with es:
            singles = es.enter_context(tc.tile_pool(name="singles", bufs=1))

            # ---- resident tensors ----
            HPs = [singles.tile([128, NTB], BF16, tag=f"hp{i}", name=f"hp{i}") for i in range(HK)]
            BHR = singles.tile([BC, T, D], BF16, tag="bhres")
            WHT = singles.tile([128, HK, H], BF16, tag="wht")
            WGT = singles.tile([128, HK, C], BF16, tag="wgt")
            WSP = singles.tile([128, HK], BF16, tag="wsp")
            BHB = singles.tile([128, HK], F32, tag="bhb")
            OHT = singles.tile([128, S, BC], BF16, tag="oht")
            Bb = singles.tile([1, C], BF16, tag="bg")
            ONESR = singles.tile([1, 128], BF16, tag="ones")
            IDBF = singles.tile([128, 128], BF16, tag="idbf")
            IDHF = singles.tile([128, 128], F32, tag="idhf")
            ESB = singles.tile([BC, T], F32, tag="esb")
            SUMS = singles.tile([BC, NE8], F32, tag="sums")
            RS = singles.tile([BC, 1], F32, tag="rs")
            CS = singles.tile([BC, H], F32, tag="cstate")

            nc.sync.dma_start(out=BHR, in_=d_bhres)
            for k in range(HK):
                nc.sync.dma_start(out=WHT[:, k, :], in_=d_wht[k])
                nc.sync.dma_start(out=WGT[:, k, :], in_=d_wgt[k])
            nc.sync.dma_start(out=WSP, in_=d_wsp)
            nc.sync.dma_start(out=BHB, in_=d_bhb)
            nc.sync.dma_start(out=OHT, in_=d_oht)
            nc.sync.dma_start(out=Bb, in_=d_bg)
            nc.sync.dma_start(out=ONESR, in_=d_ones)
            nc.sync.dma_start(out=IDBF, in_=d_idbf)
            nc.sync.dma_start(out=IDHF, in_=d_idhf)

            nc.vector.memset(CS, 0.0)

            # ---- preamble: Hp = batch_H @ Wi.T (+bh), into [h, (t,b)] ----
            with tc.tile_pool(name="bhtp", bufs=10) as bhtp, tc.tile_pool(
                name="hp_ps", bufs=4, space="PSUM"
            ) as hp_ps_pool:
                WIT = bhtp.tile([128, DK, H], BF16, tag="wit", bufs=1)
                for k in range(DK):
                    nc.sync.dma_start(out=WIT[:, k, :], in_=d_wit[k])
                for nb in range(NTB // 512):  # 16 blocks of 512 (t,b)
                    rhs_tiles = []
                    for kd in range(DK):
                        bt = bhtp.tile([128, 512], BF16, tag="bht_in")
                        nc.sync.dma_start(
                            out=bt,
                            in_=d_bht[kd * 128 : (kd + 1) * 128, 4 * nb : 4 * nb + 4, :],
                        )
                        rhs_tiles.append(bt)
                    for mh in range(HK):
                        ps = hp_ps_pool.tile([128, 512], F32, tag="hp_ps")
                        for kd in range(DK):
                            nc.tensor.matmul(
                                ps,
                                WIT[:, kd, mh * 128 : (mh + 1) * 128],
                                rhs_tiles[kd],
                                start=(kd == 0),
                                stop=(kd == DK - 1),
                            )
                        # fold bh while copying PSUM->SBUF (bf16 out)
                        nc.vector.tensor_scalar(
                            out=HPs[mh][:, nb * 512 : (nb + 1) * 512],
                            in0=ps,
                            scalar1=BHB[:, mh : mh + 1],
                            scalar2=None,
                            op0=ALU.add,
                        )

            # ---- step-loop pools ----
            xpool = es.enter_context(tc.tile_pool(name="xpool", bufs=2))
            wstrm = es.enter_context(tc.tile_pool(name="wstrm", bufs=5))
            dpool = es.enter_context(tc.tile_pool(name="dpool", bufs=3))
            phpool = es.enter_context(tc.tile_pool(name="phpool", bufs=2))
            htpool = es.enter_context(tc.tile_pool(name="htpool", bufs=2))
            actp = es.enter_context(tc.tile_pool(name="actp", bufs=2))
            fpool = es.enter_context(tc.tile_pool(name="fpool", bufs=2))
            ctxp = es.enter_context(tc.tile_pool(name="ctxp", bufs=2))
            xtp = es.enter_context(tc.tile_pool(name="xtp", bufs=1))

            e_psp = es.enter_context(tc.tile_pool(name="e_ps", bufs=2, space="PSUM"))
            ctx_psp = es.enter_context(
                tc.tile_pool(name="ctx_ps", bufs=1, space="PSUM")
            )
            g_psp = es.enter_context(tc.tile_pool(name="g_ps", bufs=1, space="PSUM"))
            sm_psp = es.enter_context(tc.tile_pool(name="sm_ps", bufs=1, space="PSUM"))

            # initial ph = 0 (h0 = 0), initial hT = 0
            ph_sb = phpool.tile([128, HK, BC], BF16, tag="ph")
            nc.vector.memset(ph_sb, 0.0)
            hT = htpool.tile([128, HK, BC], BF16, tag="ht")
            nc.vector.memset(hT, 0.0)

            for s in range(S):
                # -- stream gate weights for this step (hidden under tanh) --
                wc = []
                for k in range(9):
                    wt = wstrm.tile([128, 4 * H], BF16, tag="wcat")
                    nc.sync.dma_start(out=wt, in_=d_wcat[k])
                    wc.append(wt)

                ctx_ps = ctx_psp.tile([128, D], F32, tag="ctx")


                # -- attention scores + online ctx accumulation --
                e_ps = None
                for e8 in range(NE8):  # 8 t's per eighth
                    xq = xpool.tile([128, HK, E8], BF16, tag="xq")
                    for hc in range(HK):
                        ph_b = (
                            ph_sb[:, hc, :]
                            .unsqueeze(1)
                            .broadcast_to([128, E8 // BC, BC])
                        )
                        nc.vector.tensor_tensor(
                            out=xq[:, hc, :].rearrange(
                                "p (t b) -> p t b", b=BC
                            ),
                            in0=HPs[hc][:, e8 * E8 : (e8 + 1) * E8].rearrange(
                                "p (t b) -> p t b", b=BC
                            ),
                            in1=ph_b,
                            op=ALU.add,
                        )
                    nc.scalar.activation(
                        out=xq[:, :, :], in_=xq[:, :, :], func=AF.Tanh
                    )
                    # e[:, t] columns: X-tile stationary, Ws streaming ->
                    # e lands directly as [b, t] in PSUM (no scatter needed)
                    e_ps = e_psp.tile([128, 8], F32, tag="e_ps")
                    for tl in range(8):
                        for hc in range(HK):
                            nc.tensor.matmul(
                                e_ps[:, tl : tl + 1],
                                xq[:, hc, tl * BC : (tl + 1) * BC],
                                WSP[:, hc : hc + 1],
                                start=(hc == 0),
                                stop=(hc == HK - 1),
                            )
                    # exp (PSUM->SBUF) + partial sum for this eighth
                    nc.scalar.activation(
                        out=ESB[:, e8 * 8 : e8 * 8 + 8],
                        in_=e_ps,
                        func=AF.Exp,
                        accum_out=SUMS[:, e8 : e8 + 1],
                    )
                    # all 8 diag(expe_t) in one DVE op (identity bcast x expe)
                    dg8 = dpool.tile([128, 8, 128], BF16, tag="diag")
                    nc.vector.tensor_tensor(
                        out=dg8,
                        in0=IDBF.unsqueeze(1).broadcast_to([128, 8, 128]),
                        in1=ESB[:, e8 * 8 : e8 * 8 + 8]
                        .unsqueeze(2)
                        .broadcast_to([128, 8, 128]),
                        op=ALU.mult,
                    )
                    # online ctx: += diag(expe_t) @ BH_t
                    for tl in range(8):
                        t = e8 * 8 + tl
                        nc.tensor.matmul(
                            ctx_ps,
                            dg8[:, tl, :],
                            BHR[:, t, :],
                            start=(t == 0),
                            stop=(t == T - 1),
                        )

                # -- softmax denominator -> rs = 1/sum --
                nc.vector.tensor_reduce(
                    out=RS, in_=SUMS, axis=mybir.AxisListType.X, op=ALU.add
                )
                nc.vector.reciprocal(out=RS, in_=RS)

                # -- ctx -> SBUF (normalized), transpose to [d, b] --
                ctx_sb = ctxp.tile([128, D], BF16, tag="ctx_sb")
                nc.vector.tensor_scalar(
                    out=ctx_sb,
                    in0=ctx_ps,
                    scalar1=RS,
                    scalar2=None,
                    op0=ALU.mult,
                )
                xT = xtp.tile([128, DK, BC], BF16, tag="xT")
                for md in range(DK):
                    tp = sm_psp.tile([128, 512], BF16, tag="small", name="tpb")
                    nc.tensor.transpose(
                        tp[:, 0:128], ctx_sb[:, md * 128 : (md + 1) * 128], IDBF
                    )
                    nc.vector.tensor_copy(out=xT[:, md, :], in_=tp[:, 0:128])

                # -- gates = sum_k lhsT_k.T @ wcat_k  [b, 4H(i,f,o,g)] --
                g_ps = g_psp.tile([128, 4 * H], F32, tag="gates")
                lhs = [xT[:, k, :] for k in range(DK)] + [OHT[:, s, :]] + [
                    hT[:, k, :] for k in range(HK)
                ]
                for k in range(9):
                    for ng in range(4):
                        nc.tensor.matmul(
                            g_ps[:, ng * 512 : (ng + 1) * 512],
                            lhs[k],
                            wc[k][:, ng * 512 : (ng + 1) * 512],
                            start=(k == 0),
                            stop=(k == 8),
                        )

                # -- LSTM pointwise; sigmoid via tanh --
                tifo = actp.tile([128, 3 * 512], BF16, tag="tifo", bufs=1)
                nc.scalar.activation(
                    out=tifo, in_=g_ps[:, 0 : 3 * 512], func=AF.Tanh, scale=0.5
                )
                tg = actp.tile([128, 512], BF16, tag="tg")
                nc.scalar.activation(
                    out=tg, in_=g_ps[:, 3 * 512 : 4 * 512], func=AF.Tanh
                )
                p1 = fpool.tile([128, 512], F32, tag="pw")
                nc.vector.scalar_tensor_tensor(
                    out=p1,
                    in0=tifo[:, 512:1024],
                    scalar=1.0,
                    in1=CS,
                    op0=ALU.add,
                    op1=ALU.mult,
                )
                p2 = fpool.tile([128, 512], F32, tag="pw")
                nc.vector.scalar_tensor_tensor(
                    out=p2,
                    in0=tifo[:, 0:512],
                    scalar=1.0,
                    in1=tg,
                    op0=ALU.add,
                    op1=ALU.mult,
                )
                # p1 <- p1 + p2 = 2*c_new
                nc.vector.tensor_tensor(out=p1, in0=p1, in1=p2, op=ALU.add)
                nc.vector.tensor_scalar(
                    out=CS, in0=p1, scalar1=0.5, scalar2=None, op0=ALU.mult
                )
                tc2 = actp.tile([128, 512], BF16, tag="tc2")
                nc.scalar.activation(out=tc2, in_=p1, func=AF.Tanh, scale=0.5)
                h2x2 = fpool.tile([128, 512], F32, tag="h2")
                nc.vector.scalar_tensor_tensor(
                    out=h2x2,
                    in0=tifo[:, 1024:1536],
                    scalar=1.0,
                    in1=tc2,
                    op0=ALU.add,
                    op1=ALU.mult,
                )

                # -- hT = 0.5 * h2x2.T (transpose bakes the 0.5) --
                hT = htpool.tile([128, HK, BC], BF16, tag="ht")
                for mo in range(HK):
                    tp = sm_psp.tile([128, 512], F32, tag="small")
                    nc.tensor.transpose(
                        tp[:, 0:128], h2x2[:, mo * 128 : (mo + 1) * 128], IDHF
                    )
                    nc.vector.tensor_copy(out=hT[:, mo, :], in_=tp[:, 0:128])

                # -- probs_s = h @ Wg.T + bg -> DRAM --
                pr = sm_psp.tile([128, 512], F32, tag="small")
                for k in range(HK):
                    nc.tensor.matmul(
                        pr[:, 0:C],
                        hT[:, k, :],
                        WGT[:, k, :],
                        start=(k == 0),
                        stop=False,
                    )
                nc.tensor.matmul(
                    pr[:, 0:C], ONESR, Bb, start=False, stop=True
                )
                pr_sb = ctxp.tile([128, C], F32, tag="pr_sb", name="pr_sb", bufs=1)
                nc.vector.tensor_copy(out=pr_sb, in_=pr[:, 0:C])
                nc.sync.dma_start(out=d_out[:, s, :], in_=pr_sb)

                # -- ph for next step: ph = Wh @ h, [hout, b] --
                if s + 1 < S:
                    php = sm_psp.tile([128, 512], F32, tag="small")
                    for mo in range(HK):
                        for k in range(HK):
                            nc.tensor.matmul(
                                php[:, mo * 128 : (mo + 1) * 128],
                                WHT[:, k, mo * 128 : (mo + 1) * 128],
                                hT[:, k, :],
                                start=(k == 0),
                                stop=(k == HK - 1),
                            )
                    ph_sb = phpool.tile([128, HK, BC], BF16, tag="ph")
                    nc.vector.tensor_copy(
                        out=ph_sb, in_=php.rearrange("p (k b) -> p k b", b=BC)
                    )

    nc.compile()
    return nc


def _prep(inputs):
    """Host-side layout prep (casts/transposes/onehots). Returns in_maps."""
    bf = ml_dtypes.bfloat16
    batch_H = np.asarray(inputs["batch_H"], np.float32)
    text = np.asarray(inputs["text"])
    Wi = np.asarray(inputs["Wi"], np.float32)
    Wh = np.asarray(inputs["Wh"], np.float32)
    bh = np.asarray(inputs["bh"], np.float32)
    Ws = np.asarray(inputs["Ws"], np.float32)
    Wih = np.asarray(inputs["Wih"], np.float32)
    Whh = np.asarray(inputs["Whh"], np.float32)
    bih = np.asarray(inputs["bih"], np.float32)
    bhh = np.asarray(inputs["bhh"], np.float32)
    Wg = np.asarray(inputs["Wg"], np.float32)
    bg = np.asarray(inputs["bg"], np.float32)

    bht_full = np.ascontiguousarray(batch_H.transpose(2, 1, 0)).astype(bf)  # [D,T,B]
    bhres_full = batch_H.astype(bf)  # [B,T,D]

    wit = np.ascontiguousarray(Wi.T).reshape(DK, 128, H).astype(bf)
    # hT is stored as 2*h.T (transpose can't scale); fold 0.5 into all
    # weights that consume hT
    wht = np.ascontiguousarray(0.5 * Wh.T).reshape(HK, 128, H).astype(bf)
    wgt = np.ascontiguousarray(0.5 * Wg.T).reshape(HK, 128, C).astype(bf)
    wsp = np.ascontiguousarray(Ws[0].reshape(HK, 128).T).astype(bf)  # [128, HK]
    bhb = np.ascontiguousarray(bh.reshape(HK, 128).T).astype(np.float32)

    # gate permutation: torch (i,f,g,o) -> ours (i,f,o,g)
    perm = np.concatenate(
        [np.arange(0, 1024), np.arange(1536, 2048), np.arange(1024, 1536)]
    )
    Wihp = Wih[perm]
    Whhp = Whh[perm]
    biasp = (bih + bhh)[perm]
    xmat = np.zeros((XDIM, 4 * H), np.float32)
    xmat[0:D] = Wihp[:, 0:D].T
    xmat[D : D + C] = Wihp[:, D : D + C].T
    xmat[D + C] = biasp
    wcat = np.concatenate([xmat, 0.5 * Whhp.T], axis=0)  # [1152, 2048]
    wcat = np.ascontiguousarray(wcat).reshape(9, 128, 4 * H).astype(bf)

    # one-hot (transposed, with constant-1 row at 96) per core
    oht_full = np.zeros((128, S, B), np.float32)
    cb = np.arange(C)
    for s in range(S):
        oht_full[:C, s, :] = (text[:, s][None, :] == cb[:, None]).astype(np.float32)
    oht_full[C, :, :] = 1.0
    oht_full = oht_full.astype(bf)

    bgr = bg.reshape(1, C).astype(bf)
    onesr = np.ones((1, 128), bf)
    idbf = np.eye(128, dtype=np.float32).astype(bf)
    idhf = np.eye(128, dtype=np.float32)

    in_maps = []
    for c in range(NCORES):
        sl = slice(c * BC, (c + 1) * BC)
        in_maps.append(
            {
                "bht": np.ascontiguousarray(bht_full[:, :, sl]),
                "bhres": np.ascontiguousarray(bhres_full[sl]),
                "wit": wit,
                "wcat": wcat,
                "wht": wht,
                "wgt": wgt,
                "wsp": wsp,
                "bhb": bhb,
                "oht": np.ascontiguousarray(oht_full[:, :, sl]),
                "bgr": bgr,
                "onesr": onesr,
                "idbf": idbf,
                "idhf": idhf,
            }
        )
    return in_maps


def get_nc():
    if "nc" not in _CACHE:
        _CACHE["nc"] = _build()
    return _CACHE["nc"]


def kernel(trace=False, **inputs) -> np.ndarray:
    nc = get_nc()
    in_maps = _prep(inputs)
    res = run_bass_kernel_spmd(
        nc, in_maps, core_ids=list(range(NCORES)), trace=trace
    )
    out = np.concatenate([r["probs"] for r in res.results], axis=0)
    _CACHE["last_results"] = res
    return out


# revision 16
# speedup vs baseline: 2.8098x; 1.0380x over previous
"""Trainium2 Bass kernel for attention-LSTM decoder (teacher-forced).

Reference computation (per batch element b, S=21 steps):
    Hp = batch_H @ Wi.T                      [B,T,H]   (precomputed once)
    per step s:
        ph    = h @ Wh.T + bh                [B,H]
        e     = tanh(Hp + ph[:,None,:]) @ Ws [B,T]
        alpha = softmax(e, axis=T)
        ctx   = alpha @ batch_H              [B,D]
        gates = [ctx,oh] @ Wih.T + bih + h @ Whh.T + bhh
        LSTM pointwise -> h, c
    probs = hs @ Wg.T + bg                   [B,S,C]

Sharding: data-parallel over batch (1024 -> 128 per core x 8 cores),
weights replicated, recurrence local per core.

Layouts (per core, BC=128):
  Hp   resident SBUF [h(4x128 part), t*128+b (8192 free)] bf16, +bh folded
  BH   resident SBUF [b(128 part), t, d] bf16  (ctx matmul moving operand)
  scores: Z = Hp + ph (DVE bcast add, bf16 2x) -> tanh (ACT) ->
          e = Ws.T @ X per 512-block (PE, M=1 matvec, K-chunk accumulated)
  softmax: exp on ACT with accum_out (no max subtraction; |e|<=18 so safe)
  ctx: sum_t diag(expe_t) @ BH_t accumulated in PSUM (PE), normalized by
       1/sum(expe) during PSUM->SBUF copy (per-partition scalar)
  gates: out[b, 4H] = sum_k xT/hT[k].T @ Wcat[k]; bias via constant-1 row
  sigmoid(x) = 0.5*tanh(x/2)+0.5 derived on DVE so ACT uses one table set
"""

import numpy as np
import ml_dtypes

import sys

sys.path.insert(0, "/opt/trn_rl_repo")

import concourse.bass as bass  # noqa: E402
import concourse.mybir as mybir  # noqa: E402
import concourse.tile as tile  # noqa: E402
from concourse import bacc  # noqa: E402
from concourse.bass_utils import run_bass_kernel_spmd  # noqa: E402

BF16 = mybir.dt.bfloat16
F32 = mybir.dt.float32
AF = mybir.ActivationFunctionType
ALU = mybir.AluOpType

B, T, D, H, C, S = 1024, 64, 512, 512, 96, 21
NCORES = 8
BC = B // NCORES  # 128 batch per core
HK = H // 128  # 4 h chunks
DK = D // 128  # 4 d chunks
NTB = T * BC  # 8192 flattened (t,b), t-major
XDIM = 640  # ctx(512) + onehot(96) + bias-one(1) + pad(31)
XK = XDIM // 128  # 5
NE8 = 8  # eighths of the (t,b) range per step (8 t each)
E8 = NTB // NE8  # 1024 flat elements per eighth

_CACHE = {}


def _build():
    """Build the Bass program (single NEFF, SPMD across 8 cores)."""
    nc = bacc.Bacc(
        "TRN2",
        target_bir_lowering=False,
        debug=False,
        enable_asserts=False,
        num_devices=1,
    )

    # ---- DRAM I/O (per-core shapes) ----
    d_bht = nc.dram_tensor("bht", [D, T, BC], BF16, kind="ExternalInput").ap()
    d_bhres = nc.dram_tensor("bhres", [BC, T, D], BF16, kind="ExternalInput").ap()
    d_wit = nc.dram_tensor("wit", [DK, 128, H], BF16, kind="ExternalInput").ap()
    d_wcat = nc.dram_tensor("wcat", [9, 128, 4 * H], BF16, kind="ExternalInput").ap()
    d_wht = nc.dram_tensor("wht", [HK, 128, H], BF16, kind="ExternalInput").ap()
    d_wgt = nc.dram_tensor("wgt", [HK, 128, C], BF16, kind="ExternalInput").ap()
    d_wsp = nc.dram_tensor("wsp", [128, HK], BF16, kind="ExternalInput").ap()
    d_bhb = nc.dram_tensor("bhb", [128, HK], F32, kind="ExternalInput").ap()
    d_oht = nc.dram_tensor("oht", [128, S, BC], BF16, kind="ExternalInput").ap()
    d_bg = nc.dram_tensor("bgr", [1, C], BF16, kind="ExternalInput").ap()
    d_ones = nc.dram_tensor("onesr", [1, 128], BF16, kind="ExternalInput").ap()
    d_idbf = nc.dram_tensor("idbf", [128, 128], BF16, kind="ExternalInput").ap()
    d_idhf = nc.dram_tensor("idhf", [128, 128], F32, kind="ExternalInput").ap()
    d_out = nc.dram_tensor("probs", [BC, S, C], F32, kind="ExternalOutput").ap()

    with tile.TileContext(nc) as tc:
        import contextlib

        es = contextlib.ExitStack()
        with es:
            singles = es.enter_context(tc.tile_pool(name="singles", bufs=1))

            # ---- resident tensors ----
            HPs = [singles.tile([128, NTB], BF16, tag=f"hp{i}", name=f"hp{i}") for i in range(HK)]
            BHR = singles.tile([BC, T, D], BF16, tag="bhres")
            WHT = singles.tile([128, HK, H], BF16, tag="wht")
            WGT = singles.tile([128, HK, C], BF16, tag="wgt")
            WSP = singles.tile([128, HK], BF16, tag="wsp")
            BHB = singles.tile([128, HK], F32, tag="bhb")
            OHT = singles.tile([128, S, BC], BF16, tag="oht")
            Bb = singles.tile([1, C], BF16, tag="bg")
            ONESR = singles.tile([1, 128], BF16, tag="ones")
            IDBF = singles.tile([128, 128], BF16, tag="idbf")
            IDHF = singles.tile([128, 128], F32, tag="idhf")
            ESB = singles.tile([BC, T], F32, tag="esb")
            SUMS = singles.tile([BC, NE8], F32, tag="sums")
            RS = singles.tile([BC, 1], F32, tag="rs")
            CS = singles.tile([BC, H], F32, tag="cstate")

            nc.sync.dma_start(out=BHR, in_=d_bhres)
            for k in range(HK):
                nc.sync.dma_start(out=WHT[:, k, :], in_=d_wht[k])
                nc.sync.dma_start(out=WGT[:, k, :], in_=d_wgt[k])
            nc.sync.dma_start(out=WSP, in_=d_wsp)
            nc.sync.dma_start(out=BHB, in_=d_bhb)
            nc.sync.dma_start(out=OHT, in_=d_oht)
            nc.sync.dma_start(out=Bb, in_=d_bg)
            nc.sync.dma_start(out=ONESR, in_=d_ones)
            nc.sync.dma_start(out=IDBF, in_=d_idbf)
            nc.sync.dma_start(out=IDHF, in_=d_idhf)

            nc.vector.memset(CS, 0.0)

            # ---- preamble: Hp = batch_H @ Wi.T (+bh), into [h, (t,b)] ----
            with tc.tile_pool(name="bhtp", bufs=10) as bhtp, tc.tile_pool(
                name="hp_ps", bufs=4, space="PSUM"
            ) as hp_ps_pool:
                WIT = bhtp.tile([128, DK, H], BF16, tag="wit", bufs=1)
                for k in range(DK):
                    nc.sync.dma_start(out=WIT[:, k, :], in_=d_wit[k])
                for nb in range(NTB // 512):  # 16 blocks of 512 (t,b)
                    rhs_tiles = []
                    for kd in range(DK):
                        bt = bhtp.tile([128, 512], BF16, tag="bht_in")
                        nc.sync.dma_start(
                            out=bt,
                            in_=d_bht[kd * 128 : (kd + 1) * 128, 4 * nb : 4 * nb + 4, :],
                        )
                        rhs_tiles.append(bt)
                    for mh in range(HK):
                        ps = hp_ps_pool.tile([128, 512], F32, tag="hp_ps")
                        for kd in range(DK):
                            nc.tensor.matmul(
                                ps,
                                WIT[:, kd, mh * 128 : (mh + 1) * 128],
                                rhs_tiles[kd],
                                start=(kd == 0),
                                stop=(kd == DK - 1),
                            )
                        # fold bh while copying PSUM->SBUF (bf16 out)
                        nc.vector.tensor_scalar(
                            out=HPs[mh][:, nb * 512 : (nb + 1) * 512],
                            in0=ps,
                            scalar1=BHB[:, mh : mh + 1],
                            scalar2=None,
                            op0=ALU.add,
                        )

            # ---- step-loop pools ----
            xpool = es.enter_context(tc.tile_pool(name="xpool", bufs=2))
            wstrm = es.enter_context(tc.tile_pool(name="wstrm", bufs=5))
            dpool = es.enter_context(tc.tile_pool(name="dpool", bufs=3))
            phpool = es.enter_context(tc.tile_pool(name="phpool", bufs=2))
            htpool = es.enter_context(tc.tile_pool(name="htpool", bufs=2))
            actp = es.enter_context(tc.tile_pool(name="actp", bufs=2))
            fpool = es.enter_context(tc.tile_pool(name="fpool", bufs=2))
            ctxp = es.enter_context(tc.tile_pool(name="ctxp", bufs=2))
            xtp = es.enter_context(tc.tile_pool(name="xtp", bufs=1))

            e_psp = es.enter_context(tc.tile_pool(name="e_ps", bufs=2, space="PSUM"))
            ctx_psp = es.enter_context(
                tc.tile_pool(name="ctx_ps", bufs=1, space="PSUM")
            )
            g_psp = es.enter_context(tc.tile_pool(name="g_ps", bufs=1, space="PSUM"))
            sm_psp = es.enter_context(tc.tile_pool(name="sm_ps", bufs=1, space="PSUM"))

            # initial ph = 0 (h0 = 0), initial hT = 0
            ph_sb = phpool.tile([128, HK, BC], BF16, tag="ph")
            nc.vector.memset(ph_sb, 0.0)
            hT = htpool.tile([128, HK, BC], BF16, tag="ht")
            nc.vector.memset(hT, 0.0)

            for s in range(S):
                # -- stream gate weights for this step (hidden under tanh) --
                wc = []
                for k in range(9):
                    wt = wstrm.tile([128, 4 * H], BF16, tag="wcat")
                    nc.sync.dma_start(out=wt, in_=d_wcat[k])
                    wc.append(wt)

                ctx_ps = ctx_psp.tile([128, D], F32, tag="ctx")


                # -- attention scores + online ctx accumulation --
                # software-pipelined: exp/diag/ctx of eighth k are emitted
                # after tanh of eighth k+1 so e-matmul latency never stalls
                # the ACT tanh stream (in-order engine)
                eq = []  # pending (e8, e_ps)

                def flush_eighth(e8, e_ps):
                    nc.scalar.activation(
                        out=ESB[:, e8 * 8 : e8 * 8 + 8],
                        in_=e_ps,
                        func=AF.Exp,
                        accum_out=SUMS[:, e8 : e8 + 1],
                    )
                    dg8 = dpool.tile(
                        [128, 8, 128], BF16, tag="diag", name="dg8"
                    )
                    nc.vector.tensor_tensor(
                        out=dg8,
                        in0=IDBF.unsqueeze(1).broadcast_to([128, 8, 128]),
                        in1=ESB[:, e8 * 8 : e8 * 8 + 8]
                        .unsqueeze(2)
                        .broadcast_to([128, 8, 128]),
                        op=ALU.mult,
                    )
                    for tl in range(8):
                        t = e8 * 8 + tl
                        nc.tensor.matmul(
                            ctx_ps,
                            dg8[:, tl, :],
                            BHR[:, t, :],
                            start=(t == 0),
                            stop=(t == T - 1),
                        )

                for e8 in range(NE8):  # 8 t's per eighth
                    xq = xpool.tile([128, HK, E8], BF16, tag="xq")
                    for hc in range(HK):
                        ph_b = (
                            ph_sb[:, hc, :]
                            .unsqueeze(1)
                            .broadcast_to([128, E8 // BC, BC])
                        )
                        nc.vector.tensor_tensor(
                            out=xq[:, hc, :].rearrange(
                                "p (t b) -> p t b", b=BC
                            ),
                            in0=HPs[hc][:, e8 * E8 : (e8 + 1) * E8].rearrange(
                                "p (t b) -> p t b", b=BC
                            ),
                            in1=ph_b,
                            op=ALU.add,
                        )
                    nc.scalar.activation(
                        out=xq[:, :, :], in_=xq[:, :, :], func=AF.Tanh
                    )
                    if eq:
                        flush_eighth(*eq.pop())
                    # e[:, t] columns: X-tile stationary, Ws streaming ->
                    # e lands directly as [b, t] in PSUM (no scatter)
                    e_ps = e_psp.tile([128, 8], F32, tag="e_ps")
                    for tl in range(8):
                        for hc in range(HK):
                            nc.tensor.matmul(
                                e_ps[:, tl : tl + 1],
                                xq[:, hc, tl * BC : (tl + 1) * BC],
                                WSP[:, hc : hc + 1],
                                start=(hc == 0),
                                stop=(hc == HK - 1),
                            )
                    eq.append((e8, e_ps))
                flush_eighth(*eq.pop())

                # -- softmax denominator -> rs = 1/sum --
                nc.vector.tensor_reduce(
                    out=RS, in_=SUMS, axis=mybir.AxisListType.X, op=ALU.add
                )
                nc.vector.reciprocal(out=RS, in_=RS)

                # -- ctx -> SBUF (normalized), transpose to [d, b] --
                ctx_sb = ctxp.tile([128, D], BF16, tag="ctx_sb")
                nc.vector.tensor_scalar(
                    out=ctx_sb,
                    in0=ctx_ps,
                    scalar1=RS,
                    scalar2=None,
                    op0=ALU.mult,
                )
                xT = xtp.tile([128, DK, BC], BF16, tag="xT")
                for md in range(DK):
                    tp = sm_psp.tile([128, 512], BF16, tag="small", name="tpb")
                    nc.tensor.transpose(
                        tp[:, 0:128], ctx_sb[:, md * 128 : (md + 1) * 128], IDBF
                    )
                    nc.vector.tensor_copy(out=xT[:, md, :], in_=tp[:, 0:128])

                # -- gates = sum_k lhsT_k.T @ wcat_k  [b, 4H(i,f,o,g)] --
                g_ps = g_psp.tile([128, 4 * H], F32, tag="gates")
                lhs = [xT[:, k, :] for k in range(DK)] + [OHT[:, s, :]] + [
                    hT[:, k, :] for k in range(HK)
                ]
                for k in range(9):
                    for ng in range(4):
                        nc.tensor.matmul(
                            g_ps[:, ng * 512 : (ng + 1) * 512],
                            lhs[k],
                            wc[k][:, ng * 512 : (ng + 1) * 512],
                            start=(k == 0),
                            stop=(k == 8),
                        )

                # -- LSTM pointwise; sigmoid via tanh --
                tifo = actp.tile([128, 3 * 512], BF16, tag="tifo", bufs=1)
                # f first so p1 can start while i/o still activating
                nc.scalar.activation(
                    out=tifo[:, 512:1024],
                    in_=g_ps[:, 512:1024],
                    func=AF.Tanh,
                    scale=0.5,
                )
                p1 = fpool.tile([128, 512], F32, tag="pw")
                nc.vector.scalar_tensor_tensor(
                    out=p1,
                    in0=tifo[:, 512:1024],
                    scalar=1.0,
                    in1=CS,
                    op0=ALU.add,
                    op1=ALU.mult,
                )
                nc.scalar.activation(
                    out=tifo[:, 0:512],
                    in_=g_ps[:, 0:512],
                    func=AF.Tanh,
                    scale=0.5,
                )
                tg = actp.tile([128, 512], BF16, tag="tg")
                nc.scalar.activation(
                    out=tg, in_=g_ps[:, 3 * 512 : 4 * 512], func=AF.Tanh
                )
                nc.scalar.activation(
                    out=tifo[:, 1024:1536],
                    in_=g_ps[:, 1024:1536],
                    func=AF.Tanh,
                    scale=0.5,
                )
                p2 = fpool.tile([128, 512], F32, tag="pw")
                nc.vector.scalar_tensor_tensor(
                    out=p2,
                    in0=tifo[:, 0:512],
                    scalar=1.0,
                    in1=tg,
                    op0=ALU.add,
                    op1=ALU.mult,
                )
                # p1 <- p1 + p2 = 2*c_new
                nc.vector.tensor_tensor(out=p1, in0=p1, in1=p2, op=ALU.add)
                nc.vector.tensor_scalar(
                    out=CS, in0=p1, scalar1=0.5, scalar2=None, op0=ALU.mult
                )
                tc2 = actp.tile([128, 512], BF16, tag="tc2")
                nc.scalar.activation(out=tc2, in_=p1, func=AF.Tanh, scale=0.5)
                h2x2 = fpool.tile([128, 512], F32, tag="h2")
                nc.vector.scalar_tensor_tensor(
                    out=h2x2,
                    in0=tifo[:, 1024:1536],
                    scalar=1.0,
                    in1=tc2,
                    op0=ALU.add,
                    op1=ALU.mult,
                )

                # -- hT = 0.5 * h2x2.T (transpose bakes the 0.5) --
                hT = htpool.tile([128, HK, BC], BF16, tag="ht")
                for mo in range(HK):
                    tp = sm_psp.tile([128, 512], F32, tag="small")
                    nc.tensor.transpose(
                        tp[:, 0:128], h2x2[:, mo * 128 : (mo + 1) * 128], IDHF
                    )
                    nc.vector.tensor_copy(out=hT[:, mo, :], in_=tp[:, 0:128])

                # -- ph for next step first (critical path): ph = Wh @ h --
                if s + 1 < S:
                    php = sm_psp.tile([128, 512], F32, tag="small")
                    for mo in range(HK):
                        for k in range(HK):
                            nc.tensor.matmul(
                                php[:, mo * 128 : (mo + 1) * 128],
                                WHT[:, k, mo * 128 : (mo + 1) * 128],
                                hT[:, k, :],
                                start=(k == 0),
                                stop=(k == HK - 1),
                            )
                    ph_sb = phpool.tile([128, HK, BC], BF16, tag="ph")
                    nc.vector.tensor_copy(
                        out=ph_sb, in_=php.rearrange("p (k b) -> p k b", b=BC)
                    )

                # -- probs_s = h @ Wg.T + bg -> DRAM (off critical path) --
                pr = sm_psp.tile([128, 512], F32, tag="small")
                for k in range(HK):
                    nc.tensor.matmul(
                        pr[:, 0:C],
                        hT[:, k, :],
                        WGT[:, k, :],
                        start=(k == 0),
                        stop=False,
                    )
                nc.tensor.matmul(
                    pr[:, 0:C], ONESR, Bb, start=False, stop=True
                )
                pr_sb = ctxp.tile([128, C], F32, tag="pr_sb", name="pr_sb", bufs=1)
                nc.vector.tensor_copy(out=pr_sb, in_=pr[:, 0:C])
                nc.sync.dma_start(out=d_out[:, s, :], in_=pr_sb)

    nc.compile()
    return nc


def _prep(inputs):
    """Host-side layout prep (casts/transposes/onehots). Returns in_maps."""
    bf = ml_dtypes.bfloat16
    batch_H = np.asarray(inputs["batch_H"], np.float32)
    text = np.asarray(inputs["text"])
    Wi = np.asarray(inputs["Wi"], np.float32)
    Wh = np.asarray(inputs["Wh"], np.float32)
    bh = np.asarray(inputs["bh"], np.float32)
    Ws = np.asarray(inputs["Ws"], np.float32)
    Wih = np.asarray(inputs["Wih"], np.float32)
    Whh = np.asarray(inputs["Whh"], np.float32)
    bih = np.asarray(inputs["bih"], np.float32)
    bhh = np.asarray(inputs["bhh"], np.float32)
    Wg = np.asarray(inputs["Wg"], np.float32)
    bg = np.asarray(inputs["bg"], np.float32)

    bht_full = np.ascontiguousarray(batch_H.transpose(2, 1, 0)).astype(bf)  # [D,T,B]
    bhres_full = batch_H.astype(bf)  # [B,T,D]

    wit = np.ascontiguousarray(Wi.T).reshape(DK, 128, H).astype(bf)
    # hT is stored as 2*h.T (transpose can't scale); fold 0.5 into all
    # weights that consume hT
    wht = np.ascontiguousarray(0.5 * Wh.T).reshape(HK, 128, H).astype(bf)
    wgt = np.ascontiguousarray(0.5 * Wg.T).reshape(HK, 128, C).astype(bf)
    wsp = np.ascontiguousarray(Ws[0].reshape(HK, 128).T).astype(bf)  # [128, HK]
    bhb = np.ascontiguousarray(bh.reshape(HK, 128).T).astype(np.float32)

    # gate permutation: torch (i,f,g,o) -> ours (i,f,o,g)
    perm = np.concatenate(
        [np.arange(0, 1024), np.arange(1536, 2048), np.arange(1024, 1536)]
    )
    Wihp = Wih[perm]
    Whhp = Whh[perm]
    biasp = (bih + bhh)[perm]
    xmat = np.zeros((XDIM, 4 * H), np.float32)
    xmat[0:D] = Wihp[:, 0:D].T
    xmat[D : D + C] = Wihp[:, D : D + C].T
    xmat[D + C] = biasp
    wcat = np.concatenate([xmat, 0.5 * Whhp.T], axis=0)  # [1152, 2048]
    wcat = np.ascontiguousarray(wcat).reshape(9, 128, 4 * H).astype(bf)

    # one-hot (transposed, with constant-1 row at 96) per core
    oht_full = np.zeros((128, S, B), np.float32)
    cb = np.arange(C)
    for s in range(S):
        oht_full[:C, s, :] = (text[:, s][None, :] == cb[:, None]).astype(np.float32)
    oht_full[C, :, :] = 1.0
    oht_full = oht_full.astype(bf)

    bgr = bg.reshape(1, C).astype(bf)
    onesr = np.ones((1, 128), bf)
    idbf = np.eye(128, dtype=np.float32).astype(bf)
    idhf = np.eye(128, dtype=np.float32)

    in_maps = []
    for c in range(NCORES):
        sl = slice(c * BC, (c + 1) * BC)
        in_maps.append(
            {
                "bht": np.ascontiguousarray(bht_full[:, :, sl]),
                "bhres": np.ascontiguousarray(bhres_full[sl]),
                "wit": wit,
                "wcat": wcat,
                "wht": wht,
                "wgt": wgt,
                "wsp": wsp,
                "bhb": bhb,
                "oht": np.ascontiguousarray(oht_full[:, :, sl]),
                "bgr": bgr,
                "onesr": onesr,
                "idbf": idbf,
                "idhf": idhf,
            }
        )
    return in_maps


def get_nc():
    if "nc" not in _CACHE:
        _CACHE["nc"] = _build()
    return _CACHE["nc"]


def kernel(trace=False, **inputs) -> np.ndarray:
    nc = get_nc()
    in_maps = _prep(inputs)
    res = run_bass_kernel_spmd(
        nc, in_maps, core_ids=list(range(NCORES)), trace=trace
    )
    out = np.concatenate([r["probs"] for r in res.results], axis=0)
    _CACHE["last_results"] = res
    return out


# revision 18
# speedup vs baseline: 2.9418x; 1.0470x over previous
"""Trainium2 Bass kernel for attention-LSTM decoder (teacher-forced).

Reference computation (per batch element b, S=21 steps):
    Hp = batch_H @ Wi.T                      [B,T,H]   (precomputed once)
    per step s:
        ph    = h @ Wh.T + bh                [B,H]
        e     = tanh(Hp + ph[:,None,:]) @ Ws [B,T]
        alpha = softmax(e, axis=T)
        ctx   = alpha @ batch_H              [B,D]
        gates = [ctx,oh] @ Wih.T + bih + h @ Whh.T + bhh
        LSTM pointwise -> h, c
    probs = hs @ Wg.T + bg                   [B,S,C]

Sharding: data-parallel over batch (1024 -> 128 per core x 8 cores),
weights replicated, recurrence local per core.

Layouts (per core, BC=128):
  Hp   resident SBUF [h(4x128 part), t*128+b (8192 free)] bf16, +bh folded
  BH   resident SBUF [b(128 part), t, d] bf16  (ctx matmul moving operand)
  scores: Z = Hp + ph (DVE bcast add, bf16 2x) -> tanh (ACT) ->
          e = Ws.T @ X per 512-block (PE, M=1 matvec, K-chunk accumulated)
  softmax: exp on ACT with accum_out (no max subtraction; |e|<=18 so safe)
  ctx: sum_t diag(expe_t) @ BH_t accumulated in PSUM (PE), normalized by
       1/sum(expe) during PSUM->SBUF copy (per-partition scalar)
  gates: out[b, 4H] = sum_k xT/hT[k].T @ Wcat[k]; bias via constant-1 row
  sigmoid(x) = 0.5*tanh(x/2)+0.5 derived on DVE so ACT uses one table set
"""

import numpy as np
import ml_dtypes

import sys

sys.path.insert(0, "/opt/trn_rl_repo")

import concourse.bass as bass  # noqa: E402
import concourse.mybir as mybir  # noqa: E402
import concourse.tile as tile  # noqa: E402
from concourse import bacc  # noqa: E402
from concourse.bass_utils import run_bass_kernel_spmd  # noqa: E402

BF16 = mybir.dt.bfloat16
F32 = mybir.dt.float32
AF = mybir.ActivationFunctionType
ALU = mybir.AluOpType

B, T, D, H, C, S = 1024, 64, 512, 512, 96, 21
NCORES = 8
BC = B // NCORES  # 128 batch per core
HK = H // 128  # 4 h chunks
DK = D // 128  # 4 d chunks
NTB = T * BC  # 8192 flattened (t,b), t-major
XDIM = 640  # ctx(512) + onehot(96) + bias-one(1) + pad(31)
XK = XDIM // 128  # 5
NE8 = 8  # eighths of the (t,b) range per step (8 t each)
E8 = NTB // NE8  # 1024 flat elements per eighth

_CACHE = {}


def _build():
    """Build the Bass program (single NEFF, SPMD across 8 cores)."""
    nc = bacc.Bacc(
        "TRN2",
        target_bir_lowering=False,
        debug=False,
        enable_asserts=False,
        num_devices=1,
    )

    # ---- DRAM I/O (per-core shapes) ----
    d_bht = nc.dram_tensor("bht", [D, T, BC], BF16, kind="ExternalInput").ap()
    d_bhres = nc.dram_tensor("bhres", [BC, T, D], BF16, kind="ExternalInput").ap()
    d_wit = nc.dram_tensor("wit", [DK, 128, H], BF16, kind="ExternalInput").ap()
    d_wcat = nc.dram_tensor("wcat", [9, 128, 4 * H], BF16, kind="ExternalInput").ap()
    d_wht = nc.dram_tensor("wht", [HK, 128, H], BF16, kind="ExternalInput").ap()
    d_wgt = nc.dram_tensor("wgt", [HK, 128, C], BF16, kind="ExternalInput").ap()
    d_wsp = nc.dram_tensor("wsp", [128, HK], BF16, kind="ExternalInput").ap()
    d_bhb = nc.dram_tensor("bhb", [128, HK], F32, kind="ExternalInput").ap()
    d_oht = nc.dram_tensor("oht", [128, S, BC], BF16, kind="ExternalInput").ap()
    d_bg = nc.dram_tensor("bgr", [1, C], BF16, kind="ExternalInput").ap()
    d_ones = nc.dram_tensor("onesr", [1, 128], BF16, kind="ExternalInput").ap()
    d_idbf = nc.dram_tensor("idbf", [128, 128], BF16, kind="ExternalInput").ap()
    d_out = nc.dram_tensor("probs", [BC, S, C], F32, kind="ExternalOutput").ap()

    with tile.TileContext(nc) as tc:
        import contextlib

        es = contextlib.ExitStack()
        with es:
            singles = es.enter_context(tc.tile_pool(name="singles", bufs=1))

            # ---- resident tensors ----
            HPs = [singles.tile([128, NTB], BF16, tag=f"hp{i}", name=f"hp{i}") for i in range(HK)]
            WCAT = singles.tile([128, 9, 4 * H], BF16, tag="wcat")
            WHT = singles.tile([128, HK, H], BF16, tag="wht")
            WGT = singles.tile([128, HK, C], BF16, tag="wgt")
            WSP = singles.tile([128, HK], BF16, tag="wsp")
            BHB = singles.tile([128, HK], F32, tag="bhb")
            OHT = singles.tile([128, S, BC], BF16, tag="oht")
            Bb = singles.tile([1, C], BF16, tag="bg")
            ONESR = singles.tile([1, 128], BF16, tag="ones")
            IDBF = singles.tile([128, 128], BF16, tag="idbf")
            ESB = singles.tile([BC, T], F32, tag="esb")
            SUMS = singles.tile([BC, 4], F32, tag="sums")
            RS = singles.tile([BC, 1], F32, tag="rs")
            CS = singles.tile([BC, H], F32, tag="cstate")

            for k in range(9):
                nc.sync.dma_start(out=WCAT[:, k, :], in_=d_wcat[k])
            for k in range(HK):
                nc.sync.dma_start(out=WHT[:, k, :], in_=d_wht[k])
                nc.sync.dma_start(out=WGT[:, k, :], in_=d_wgt[k])
            nc.sync.dma_start(out=WSP, in_=d_wsp)
            nc.sync.dma_start(out=BHB, in_=d_bhb)
            nc.sync.dma_start(out=OHT, in_=d_oht)
            nc.sync.dma_start(out=Bb, in_=d_bg)
            nc.sync.dma_start(out=ONESR, in_=d_ones)
            nc.sync.dma_start(out=IDBF, in_=d_idbf)

            nc.vector.memset(CS, 0.0)

            # ---- preamble: Hp = batch_H @ Wi.T (+bh), into [h, (t,b)] ----
            with tc.tile_pool(name="bhtp", bufs=10) as bhtp, tc.tile_pool(
                name="hp_ps", bufs=4, space="PSUM"
            ) as hp_ps_pool:
                WIT = bhtp.tile([128, DK, H], BF16, tag="wit", bufs=1)
                for k in range(DK):
                    nc.sync.dma_start(out=WIT[:, k, :], in_=d_wit[k])
                for nb in range(NTB // 512):  # 16 blocks of 512 (t,b)
                    rhs_tiles = []
                    for kd in range(DK):
                        bt = bhtp.tile([128, 512], BF16, tag="bht_in")
                        nc.sync.dma_start(
                            out=bt,
                            in_=d_bht[kd * 128 : (kd + 1) * 128, 4 * nb : 4 * nb + 4, :],
                        )
                        rhs_tiles.append(bt)
                    for mh in range(HK):
                        ps = hp_ps_pool.tile([128, 512], F32, tag="hp_ps")
                        for kd in range(DK):
                            nc.tensor.matmul(
                                ps,
                                WIT[:, kd, mh * 128 : (mh + 1) * 128],
                                rhs_tiles[kd],
                                start=(kd == 0),
                                stop=(kd == DK - 1),
                            )
                        # fold bh while copying PSUM->SBUF (bf16 out)
                        nc.vector.tensor_scalar(
                            out=HPs[mh][:, nb * 512 : (nb + 1) * 512],
                            in0=ps,
                            scalar1=BHB[:, mh : mh + 1],
                            scalar2=None,
                            op0=ALU.add,
                        )

            # ---- step-loop pools ----
            xpool = es.enter_context(tc.tile_pool(name="xpool", bufs=2))
            bhstr = es.enter_context(tc.tile_pool(name="bhstr", bufs=3))
            dpool = es.enter_context(tc.tile_pool(name="dpool", bufs=3))
            phpool = es.enter_context(tc.tile_pool(name="phpool", bufs=2))
            htpool = es.enter_context(tc.tile_pool(name="htpool", bufs=2))
            actp = es.enter_context(tc.tile_pool(name="actp", bufs=2))
            fpool = es.enter_context(tc.tile_pool(name="fpool", bufs=2))
            ctxp = es.enter_context(tc.tile_pool(name="ctxp", bufs=2))
            xtp = es.enter_context(tc.tile_pool(name="xtp", bufs=1))

            e_psp = es.enter_context(tc.tile_pool(name="e_ps", bufs=2, space="PSUM"))
            ctx_psp = es.enter_context(
                tc.tile_pool(name="ctx_ps", bufs=1, space="PSUM")
            )
            g_psp = es.enter_context(tc.tile_pool(name="g_ps", bufs=1, space="PSUM"))
            sm_psp = es.enter_context(tc.tile_pool(name="sm_ps", bufs=1, space="PSUM"))

            # initial ph = 0 (h0 = 0), initial hT = 0
            ph_sb = phpool.tile([128, HK, BC], BF16, tag="ph")
            nc.vector.memset(ph_sb, 0.0)
            hT = htpool.tile([128, HK, BC], BF16, tag="ht")
            nc.vector.memset(hT, 0.0)

            for s in range(S):
                ctx_ps = ctx_psp.tile([128, D], F32, tag="ctx")


                # -- attention scores + online ctx accumulation --
                # quarter-granularity, software-pipelined: exp/diag/ctx of
                # quarter k emitted after tanh of quarter k+1 so e-matmul
                # latency never stalls the in-order ACT tanh stream
                QT = 16  # t's per quarter
                eq = []  # pending (q, e_ps, bh tiles)

                def flush_quarter(q, e_ps, bhtiles):
                    nc.scalar.activation(
                        out=ESB[:, q * QT : (q + 1) * QT],
                        in_=e_ps,
                        func=AF.Exp,
                        accum_out=SUMS[:, q : q + 1],
                    )
                    for half in range(2):
                        dg8 = dpool.tile(
                            [128, 8, 128], BF16, tag="diag", name="dg8"
                        )
                        nc.vector.tensor_tensor(
                            out=dg8,
                            in0=IDBF.unsqueeze(1).broadcast_to([128, 8, 128]),
                            in1=ESB[
                                :, q * QT + 8 * half : q * QT + 8 * half + 8
                            ]
                            .unsqueeze(2)
                            .broadcast_to([128, 8, 128]),
                            op=ALU.mult,
                        )
                        for tl in range(8):
                            t = q * QT + half * 8 + tl
                            nc.tensor.matmul(
                                ctx_ps,
                                dg8[:, tl, :],
                                bhtiles[half][:, tl, :],
                                start=(t == 0),
                                stop=(t == T - 1),
                            )

                for q in range(4):  # 16 t's per quarter
                    # prefetch this quarter's batch_H tiles (2 eighths)
                    bhtiles = []
                    for half in range(2):
                        bt = bhstr.tile([BC, 8, D], BF16, tag="bhs", name="bhs")
                        t0 = q * QT + half * 8
                        nc.sync.dma_start(
                            out=bt, in_=d_bhres[:, t0 : t0 + 8, :]
                        )
                        bhtiles.append(bt)
                    xq = xpool.tile([128, HK, 2048], BF16, tag="xq")
                    for hc in range(HK):
                        ph_b = (
                            ph_sb[:, hc, :]
                            .unsqueeze(1)
                            .broadcast_to([128, QT, BC])
                        )
                        nc.vector.tensor_tensor(
                            out=xq[:, hc, :].rearrange(
                                "p (t b) -> p t b", b=BC
                            ),
                            in0=HPs[hc][:, q * 2048 : (q + 1) * 2048].rearrange(
                                "p (t b) -> p t b", b=BC
                            ),
                            in1=ph_b,
                            op=ALU.add,
                        )
                    nc.scalar.activation(
                        out=xq[:, :, :], in_=xq[:, :, :], func=AF.Tanh
                    )
                    if eq:
                        flush_quarter(*eq.pop())
                    # e[:, t] columns: X-tile stationary, Ws streaming ->
                    # e lands directly as [b, t] in PSUM (no scatter)
                    e_ps = e_psp.tile([128, QT], F32, tag="e_ps")
                    for tl in range(QT):
                        for hc in range(HK):
                            nc.tensor.matmul(
                                e_ps[:, tl : tl + 1],
                                xq[:, hc, tl * BC : (tl + 1) * BC],
                                WSP[:, hc : hc + 1],
                                start=(hc == 0),
                                stop=(hc == HK - 1),
                            )
                    eq.append((q, e_ps, bhtiles))
                flush_quarter(*eq.pop())

                # -- softmax denominator -> rs = 1/sum --
                nc.vector.tensor_reduce(
                    out=RS, in_=SUMS, axis=mybir.AxisListType.X, op=ALU.add
                )
                nc.vector.reciprocal(out=RS, in_=RS)

                # -- ctx -> SBUF (normalized), transpose to [d, b] --
                ctx_sb = ctxp.tile([128, D], BF16, tag="ctx_sb")
                nc.vector.tensor_scalar(
                    out=ctx_sb,
                    in0=ctx_ps,
                    scalar1=RS,
                    scalar2=None,
                    op0=ALU.mult,
                )
                xT = xtp.tile([128, DK, BC], BF16, tag="xT")
                for md in range(DK):
                    tp = sm_psp.tile([128, 512], BF16, tag="small", name="tpb")
                    nc.tensor.transpose(
                        tp[:, 0:128], ctx_sb[:, md * 128 : (md + 1) * 128], IDBF
                    )
                    nc.vector.tensor_copy(out=xT[:, md, :], in_=tp[:, 0:128])

                # -- gates = sum_k lhsT_k.T @ wcat_k  [b, 4H(i,f,o,g)] --
                g_ps = g_psp.tile([128, 4 * H], F32, tag="gates")
                lhs = [xT[:, k, :] for k in range(DK)] + [OHT[:, s, :]] + [
                    hT[:, k, :] for k in range(HK)
                ]
                for k in range(9):
                    for ng in range(4):
                        nc.tensor.matmul(
                            g_ps[:, ng * 512 : (ng + 1) * 512],
                            lhs[k],
                            WCAT[:, k, ng * 512 : (ng + 1) * 512],
                            start=(k == 0),
                            stop=(k == 8),
                        )

                # -- LSTM pointwise; sigmoid via tanh --
                tifo = actp.tile([128, 3 * 512], BF16, tag="tifo", bufs=1)
                # f first so p1 can start while i/o still activating
                nc.scalar.activation(
                    out=tifo[:, 512:1024],
                    in_=g_ps[:, 512:1024],
                    func=AF.Tanh,
                    scale=0.5,
                )
                p1 = fpool.tile([128, 512], F32, tag="pw")
                nc.vector.scalar_tensor_tensor(
                    out=p1,
                    in0=tifo[:, 512:1024],
                    scalar=1.0,
                    in1=CS,
                    op0=ALU.add,
                    op1=ALU.mult,
                )
                nc.scalar.activation(
                    out=tifo[:, 0:512],
                    in_=g_ps[:, 0:512],
                    func=AF.Tanh,
                    scale=0.5,
                )
                tg = actp.tile([128, 512], BF16, tag="tg")
                nc.scalar.activation(
                    out=tg, in_=g_ps[:, 3 * 512 : 4 * 512], func=AF.Tanh
                )
                nc.scalar.activation(
                    out=tifo[:, 1024:1536],
                    in_=g_ps[:, 1024:1536],
                    func=AF.Tanh,
                    scale=0.5,
                )
                p2 = fpool.tile([128, 512], F32, tag="pw")
                nc.vector.scalar_tensor_tensor(
                    out=p2,
                    in0=tifo[:, 0:512],
                    scalar=1.0,
                    in1=tg,
                    op0=ALU.add,
                    op1=ALU.mult,
                )
                # p1 <- p1 + p2 = 2*c_new
                nc.vector.tensor_tensor(out=p1, in0=p1, in1=p2, op=ALU.add)
                nc.vector.tensor_scalar(
                    out=CS, in0=p1, scalar1=0.5, scalar2=None, op0=ALU.mult
                )
                tc2 = actp.tile([128, 512], BF16, tag="tc2")
                nc.scalar.activation(out=tc2, in_=p1, func=AF.Tanh, scale=0.5)
                h2x2 = fpool.tile([128, 512], BF16, tag="h2")
                nc.vector.scalar_tensor_tensor(
                    out=h2x2,
                    in0=tifo[:, 1024:1536],
                    scalar=1.0,
                    in1=tc2,
                    op0=ALU.add,
                    op1=ALU.mult,
                )

                # -- hT = 0.5 * h2x2.T (transpose bakes the 0.5) --
                hT = htpool.tile([128, HK, BC], BF16, tag="ht")
                for mo in range(HK):
                    tp = sm_psp.tile([128, 512], BF16, tag="small", name="tpb2")
                    nc.tensor.transpose(
                        tp[:, 0:128], h2x2[:, mo * 128 : (mo + 1) * 128], IDBF
                    )
                    nc.vector.tensor_copy(out=hT[:, mo, :], in_=tp[:, 0:128])

                # -- ph for next step first (critical path): ph = Wh @ h --
                if s + 1 < S:
                    php = sm_psp.tile([128, 512], F32, tag="small")
                    for mo in range(HK):
                        for k in range(HK):
                            nc.tensor.matmul(
                                php[:, mo * 128 : (mo + 1) * 128],
                                WHT[:, k, mo * 128 : (mo + 1) * 128],
                                hT[:, k, :],
                                start=(k == 0),
                                stop=(k == HK - 1),
                            )
                    ph_sb = phpool.tile([128, HK, BC], BF16, tag="ph")
                    nc.vector.tensor_copy(
                        out=ph_sb, in_=php.rearrange("p (k b) -> p k b", b=BC)
                    )

                # -- probs_s = h @ Wg.T + bg -> DRAM (off critical path) --
                pr = sm_psp.tile([128, 512], F32, tag="small")
                for k in range(HK):
                    nc.tensor.matmul(
                        pr[:, 0:C],
                        hT[:, k, :],
                        WGT[:, k, :],
                        start=(k == 0),
                        stop=False,
                    )
                nc.tensor.matmul(
                    pr[:, 0:C], ONESR, Bb, start=False, stop=True
                )
                pr_sb = ctxp.tile([128, C], F32, tag="pr_sb", name="pr_sb", bufs=1)
                nc.vector.tensor_copy(out=pr_sb, in_=pr[:, 0:C])
                nc.sync.dma_start(out=d_out[:, s, :], in_=pr_sb)

    nc.compile()
    return nc


def _prep(inputs):
    """Host-side layout prep (casts/transposes/onehots). Returns in_maps."""
    bf = ml_dtypes.bfloat16
    batch_H = np.asarray(inputs["batch_H"], np.float32)
    text = np.asarray(inputs["text"])
    Wi = np.asarray(inputs["Wi"], np.float32)
    Wh = np.asarray(inputs["Wh"], np.float32)
    bh = np.asarray(inputs["bh"], np.float32)
    Ws = np.asarray(inputs["Ws"], np.float32)
    Wih = np.asarray(inputs["Wih"], np.float32)
    Whh = np.asarray(inputs["Whh"], np.float32)
    bih = np.asarray(inputs["bih"], np.float32)
    bhh = np.asarray(inputs["bhh"], np.float32)
    Wg = np.asarray(inputs["Wg"], np.float32)
    bg = np.asarray(inputs["bg"], np.float32)

    bht_full = np.ascontiguousarray(batch_H.transpose(2, 1, 0)).astype(bf)  # [D,T,B]
    bhres_full = batch_H.astype(bf)  # [B,T,D]

    wit = np.ascontiguousarray(Wi.T).reshape(DK, 128, H).astype(bf)
    # hT is stored as 2*h.T (transpose can't scale); fold 0.5 into all
    # weights that consume hT
    wht = np.ascontiguousarray(0.5 * Wh.T).reshape(HK, 128, H).astype(bf)
    wgt = np.ascontiguousarray(0.5 * Wg.T).reshape(HK, 128, C).astype(bf)
    wsp = np.ascontiguousarray(Ws[0].reshape(HK, 128).T).astype(bf)  # [128, HK]
    bhb = np.ascontiguousarray(bh.reshape(HK, 128).T).astype(np.float32)

    # gate permutation: torch (i,f,g,o) -> ours (i,f,o,g)
    perm = np.concatenate(
        [np.arange(0, 1024), np.arange(1536, 2048), np.arange(1024, 1536)]
    )
    Wihp = Wih[perm]
    Whhp = Whh[perm]
    biasp = (bih + bhh)[perm]
    xmat = np.zeros((XDIM, 4 * H), np.float32)
    xmat[0:D] = Wihp[:, 0:D].T
    xmat[D : D + C] = Wihp[:, D : D + C].T
    xmat[D + C] = biasp
    wcat = np.concatenate([xmat, 0.5 * Whhp.T], axis=0)  # [1152, 2048]
    wcat = np.ascontiguousarray(wcat).reshape(9, 128, 4 * H).astype(bf)

    # one-hot (transposed, with constant-1 row at 96) per core
    oht_full = np.zeros((128, S, B), np.float32)
    cb = np.arange(C)
    for s in range(S):
        oht_full[:C, s, :] = (text[:, s][None, :] == cb[:, None]).astype(np.float32)
    oht_full[C, :, :] = 1.0
    oht_full = oht_full.astype(bf)

    bgr = bg.reshape(1, C).astype(bf)
    onesr = np.ones((1, 128), bf)
    idbf = np.eye(128, dtype=np.float32).astype(bf)

    in_maps = []
    for c in range(NCORES):
        sl = slice(c * BC, (c + 1) * BC)
        in_maps.append(
            {
                "bht": np.ascontiguousarray(bht_full[:, :, sl]),
                "bhres": np.ascontiguousarray(bhres_full[sl]),
                "wit": wit,
                "wcat": wcat,
                "wht": wht,
                "wgt": wgt,
                "wsp": wsp,
                "bhb": bhb,
                "oht": np.ascontiguousarray(oht_full[:, :, sl]),
                "bgr": bgr,
                "onesr": onesr,
                "idbf": idbf,
            }
        )
    return in_maps


def get_nc():
    if "nc" not in _CACHE:
        _CACHE["nc"] = _build()
    return _CACHE["nc"]


def kernel(trace=False, **inputs) -> np.ndarray:
    nc = get_nc()
    in_maps = _prep(inputs)
    res = run_bass_kernel_spmd(
        nc, in_maps, core_ids=list(range(NCORES)), trace=trace
    )
    out = np.concatenate([r["probs"] for r in res.results], axis=0)
    _CACHE["last_results"] = res
    return out
